# revision 22
# baseline (speedup 1.0000x reference)
"""Trainium2 Bass kernel for ObservationEmbedding.

Computes, for features [B=16384, T=64, F=16] (f32), W [128, 16], b [128],
positions [64] (int32 days):

    out[b, t, 0:128]   = features[b, t, :] @ W.T + b
    out[b, t, 128:256] = pe[t]        (sinusoidal day encoding, broadcast over B)

Sharding: data-parallel over the batch dim across 8 NeuronCores. W / b /
positions (and the input-independent sin/cos day table + identity) are
replicated.

Per-core layout (rows = flattened (b, t), 131072 rows/core):
  - rows are grouped 8192 at a time; SBUF output tile O is [128 part, 64*256]
    where partition p holds rows r0 + 64*p .. r0 + 64*p + 63 (64 rows x 256
    f32 contiguous) -> the 8 MiB store DMA per group is fully contiguous.
  - features for a group load contiguously as [128, 1024] (64 rows x 16 f32
    per partition), get PE-transposed 128x128 blocks at a time, and a single
    K=128 fp32 matmul with a block-diagonal weight table computes 4 row-
    groups x 128 outputs at once (N=512).  Bias is fused into the PSUM->O
    copy as a tensor-tensor add against a broadcast bias tile.
  - because row % 64 == (64*p + k) % 64 == k, the PE half of O is the same
    for every group and every partition: it is written once at setup (device
    gather of the day table with `positions` as indices, bounced to DRAM,
    then replicated to all partitions by a partition-step-0 broadcast DMA)
    and never touched again; the store DMA re-reads it each group.
"""

import numpy as np
from contextlib import ExitStack

import concourse.bass as bass
import concourse.bacc as bacc
import concourse.mybir as mybir
import concourse.tile as tile
from concourse.bass_utils import run_bass_kernel_spmd
from concourse.masks import make_identity

F32 = mybir.dt.float32
I32 = mybir.dt.int32

N_CORES = 8
B, T, F, DH = 16384, 64, 16, 128
D = 2 * DH                       # 256
N_DAYS = 365                     # positions in [0, 365)
RPP = 64                         # rows per partition per group (must be = T)
GROUP = 128 * RPP                # 8192 rows per group
ROWS_PER_CORE = (B // N_CORES) * T   # 131072


def pe_day_table() -> np.ndarray:
    """Input-independent [365, 128] table: row d = interleaved
    sin/cos(d * inv_freq).  Mirrors reference arithmetic in float32."""
    i_even = np.arange(0, DH, 2, dtype=np.float32)
    inv_freq = np.power(
        np.float32(10000.0), -(np.float32(2.0) * i_even) / np.float32(DH)
    ).astype(np.float32)
    days = np.arange(N_DAYS, dtype=np.float32)
    ang = (days[:, None] * inv_freq[None, :]).astype(np.float32)
    tab = np.empty((N_DAYS, DH), dtype=np.float32)
    tab[:, 0::2] = np.sin(ang)
    tab[:, 1::2] = np.cos(ang)
    return tab


def build_program(rows: int = ROWS_PER_CORE):
    assert rows % GROUP == 0
    n_groups = rows // GROUP

    nc = bacc.Bacc("TRN2", target_bir_lowering=False, debug=False)

    feat = nc.dram_tensor("features", [rows, F], F32, kind="ExternalInput").ap()
    w_in = nc.dram_tensor("W", [DH, F], F32, kind="ExternalInput").ap()
    b_in = nc.dram_tensor("b", [DH], F32, kind="ExternalInput").ap()
    pos_in = nc.dram_tensor("positions", [T], I32, kind="ExternalInput").ap()
    tab_in = nc.dram_tensor("petab", [N_DAYS, DH], F32, kind="ExternalInput").ap()
    out = nc.dram_tensor("out", [rows, D], F32, kind="ExternalOutput").ap()
    pe_dram = nc.dram_tensor("pe_dram", [T * DH], F32).ap()  # internal scratch

    with tile.TileContext(nc) as tc, ExitStack() as ctx:
        const = ctx.enter_context(tc.tile_pool(name="const", bufs=1))
        psum = ctx.enter_context(tc.tile_pool(name="psum", bufs=1, space="PSUM"))
        ftp = ctx.enter_context(tc.tile_pool(name="ftp", bufs=6))
        tsp = ctx.enter_context(tc.tile_pool(name="tsp", bufs=3))
        opool = ctx.enter_context(tc.tile_pool(name="opool", bufs=1))

        feat_r = feat.rearrange("(g p r) f -> g p (r f)", p=128, r=RPP)
        out_r = out.rearrange("(g p r) d -> g p (r d)", p=128, r=RPP)

        # ---------- setup: constants ----------
        # PE warmup: a few junk matmuls issued at t=0 keep the PE busy long
        # enough (~3.4us) that the HAM clock-gate opens to 2.4 GHz before the
        # first real matmuls arrive (outputs are never read; memory writes are
        # not DCE'd).
        warm_src = const.tile([128, 512], F32)
        nc.vector.memset(warm_src[:], 0.0)
        for wi in range(6):
            warm_ps = psum.tile([128, 512], F32, tag="mm", bufs=5, name="warm_ps")
            nc.tensor.matmul(
                warm_ps[:],
                lhsT=warm_src[:, 0:128],
                rhs=warm_src[:],
                start=True,
                stop=True,
            )

        # issue the first feature load immediately so it overlaps setup
        ft0 = ftp.tile([128, RPP * F], F32, tag="ft", name="ft")
        nc.scalar.dma_start(ft0[:], feat_r[0])

        pos_sb = const.tile([T, 1], I32)
        nc.sync.dma_start(pos_sb[:], pos_in[:])
        w_sb = const.tile([DH, F], F32)
        nc.sync.dma_start(w_sb[:], w_in[:])
        b_sb = const.tile([1, DH], F32)
        nc.scalar.dma_start(b_sb[:], b_in[:])

        # gather pe[t] = petab[positions[t]] on device (embedding lookup),
        # then bounce through DRAM so a broadcast (partition-step-0) DMA can
        # replicate it to all 128 partitions of the staging buffers; first in
        # gpsimd program order so the chain starts as soon as pos_sb lands
        pe64 = const.tile([T, DH], F32)
        nc.gpsimd.indirect_dma_start(
            out=pe64[:],
            out_offset=None,
            in_=tab_in[:],
            in_offset=bass.IndirectOffsetOnAxis(ap=pos_sb[:, :1], axis=0),
        )
        nc.gpsimd.dma_start(pe_dram[:], pe64[:])

        ident = const.tile([128, 128], F32)
        make_identity(nc, ident)
        ones1 = const.tile([1, 128], F32)
        nc.gpsimd.memset(ones1[:], 1.0)

        # W [128,16] -> Wt [16,128] via PE transpose
        wt_ps = psum.tile([F, 128], F32, tag="tr", bufs=2)
        nc.tensor.transpose(wt_ps[:], w_sb[:], ident[:])
        wt_sb = const.tile([F, 128], F32)
        nc.vector.tensor_copy(wt_sb[:], wt_ps[:])

        # Block-diagonal weight tables [128, 512]:
        #   wta[16*j + f, 128*j + d] = Wt[f, d]   (row-groups 0..3)
        #   wtb[16*(j+4) + f, 128*j + d] = Wt[f, d]   (row-groups 4..7)
        wta = const.tile([128, 4 * DH], F32)
        wtb = const.tile([128, 4 * DH], F32)
        nc.vector.memset(wta[:], 0.0)
        nc.vector.memset(wtb[:], 0.0)
        # spread the partition-moving scatter copies over both HWDGE queues
        for j in range(4):
            nc.sync.dma_start(
                wta[16 * j : 16 * (j + 1), DH * j : DH * (j + 1)], wt_sb[:]
            )
            nc.scalar.dma_start(
                wtb[16 * (j + 4) : 16 * (j + 5), DH * j : DH * (j + 1)], wt_sb[:]
            )

        # bias broadcast [128, 512] (b tiled 4x along free dim)
        bias_ps = psum.tile([128, DH], F32, tag="tr", bufs=2)
        nc.tensor.matmul(bias_ps[:], lhsT=ones1[:], rhs=b_sb[:], start=True, stop=True)
        biasb = const.tile([128, 4 * DH], F32)
        for j in range(4):
            nc.vector.tensor_copy(biasb[:, DH * j : DH * (j + 1)], bias_ps[:])
        biasb3 = biasb.rearrange("p (j d) -> p j d", d=DH)

        # persistent output staging buffers (ping-pong); the pe columns are
        # written once (interleaved with group 0 below) and never re-written
        o_bufs = []
        for oi in range(2):
            o_t = opool.tile([128, RPP * D], F32, tag=f"o{oi}", name=f"obuf{oi}")
            o_bufs.append(o_t)

        # pe-column prefill: broadcast-DMA the gathered pe row block from DRAM
        # to every partition of O0.  These go on the gpsimd (SWDGE) queue,
        # which is otherwise idle — on the sync queue they would head-of-line
        # block group 0's piece stores (HWDGE queues drain FIFO).  O1's pe
        # columns are filled by same-partition DVE copies from O0 inside the
        # group-0 loop below.
        o_pe = [
            ob.rearrange("p (k d) -> p k d", d=D)[:, :, DH:D] for ob in o_bufs
        ]
        KQ = RPP // 4  # 16 k-blocks per prefill piece
        for q in range(4):
            src = bass.AP(
                pe_dram.tensor,
                KQ * DH * q,
                [[0, 128], [1, KQ * DH]],
            )
            nc.gpsimd.dma_start(o_pe[0][:, KQ * q : KQ * (q + 1), :], src)

        # ---------- main loop ----------
        # group 0's 8 MiB store is split into 4 pieces of 2 MiB so the store
        # pipeline starts as soon as the first quarter is ready; later groups
        # use 2 pieces of 4 MiB (with sub-tile WAR tracking, the next group's
        # compute into a piece starts as soon as that piece's store drained).
        for g in range(n_groups):
            o_t = o_bufs[g % 2]
            o3 = o_t.rearrange("p (k d) -> p k d", d=D)
            if g == 0:
                ft = ft0
            else:
                ft = ftp.tile([128, RPP * F], F32, tag="ft", name="ft")
                nc.scalar.dma_start(ft[:], feat_r[g])
            npiece = 4 if g == 0 else 2
            cpp = 8 // npiece  # c-blocks per piece
            for q in range(npiece):
                for c in range(cpp * q, cpp * (q + 1)):
                    tr_ps = psum.tile([128, 128], F32, tag="tr", bufs=2, name="tr_ps")
                    nc.tensor.transpose(
                        tr_ps[:], ft[:, 128 * c : 128 * (c + 1)], ident[:]
                    )
                    t_sb = tsp.tile([128, 128], F32, tag="t", name="t_sb")
                    nc.vector.tensor_copy(t_sb[:], tr_ps[:])
                    for half, wblk in ((0, wta), (1, wtb)):
                        mm_ps = psum.tile(
                            [128, 512], F32, tag="mm", bufs=5, name="mm_ps"
                        )
                        nc.tensor.matmul(
                            mm_ps[:], lhsT=t_sb[:], rhs=wblk[:], start=True, stop=True
                        )
                        dst = o3[:, 8 * c + 4 * half : 8 * c + 4 * half + 4, 0:DH]
                        nc.vector.tensor_add(
                            dst, mm_ps.rearrange("p (j d) -> p j d", d=DH), biasb3
                        )
                lo = RPP * D * q // npiece
                hi = RPP * D * (q + 1) // npiece
                nc.sync.dma_start(out_r[g][:, lo:hi], o_t[:, lo:hi])
                if g == 0:
                    # fill O1's pe columns for this piece's k-range (cheap
                    # same-partition copy; needed before group 1's store)
                    k0, k1 = KQ * q, KQ * (q + 1)
                    nc.vector.tensor_copy(
                        o_pe[1][:, k0:k1, :], o_pe[0][:, k0:k1, :]
                    )

    nc.compile()
    return nc


_NC_CACHE: dict[int, object] = {}


def _get_program(rows: int):
    if rows not in _NC_CACHE:
        _NC_CACHE[rows] = build_program(rows)
    return _NC_CACHE[rows]


def kernel(features, W, b, positions, trace: bool = False, **run_kwargs):
    features = np.ascontiguousarray(np.asarray(features, dtype=np.float32))
    W = np.ascontiguousarray(np.asarray(W, dtype=np.float32))
    b = np.ascontiguousarray(np.asarray(b, dtype=np.float32))
    positions = np.ascontiguousarray(np.asarray(positions, dtype=np.int32))
    assert features.shape == (B, T, F)

    nc = _get_program(ROWS_PER_CORE)
    tab = pe_day_table()
    shards = features.reshape(N_CORES, ROWS_PER_CORE, F)
    in_maps = [
        {
            "features": shards[i],
            "W": W,
            "b": b,
            "positions": positions,
            "petab": tab,
        }
        for i in range(N_CORES)
    ]
    res = run_bass_kernel_spmd(
        nc, in_maps, core_ids=list(range(N_CORES)), trace=trace, **run_kwargs
    )
    out = np.concatenate(
        [res.results[i]["out"].reshape(B // N_CORES, T, D) for i in range(N_CORES)],
        axis=0,
    )
    if trace:
        kernel.last_results = res
    return out


# revision 24
# speedup vs baseline: 1.0142x; 1.0142x over previous
"""Trainium2 Bass kernel for ObservationEmbedding.

Computes, for features [B=16384, T=64, F=16] (f32), W [128, 16], b [128],
positions [64] (int32 days):

    out[b, t, 0:128]   = features[b, t, :] @ W.T + b
    out[b, t, 128:256] = pe[t]        (sinusoidal day encoding, broadcast over B)

Sharding: data-parallel over the batch dim across 8 NeuronCores. W / b /
positions (and the input-independent sin/cos day table + identity) are
replicated.

Per-core layout (rows = flattened (b, t), 131072 rows/core):
  - rows are grouped 8192 at a time; SBUF output tile O is [128 part, 64*256]
    where partition p holds rows r0 + 64*p .. r0 + 64*p + 63 (64 rows x 256
    f32 contiguous) -> the 8 MiB store DMA per group is fully contiguous.
  - features for a group load contiguously as [128, 1024] (64 rows x 16 f32
    per partition), get PE-transposed 128x128 blocks at a time, and a single
    K=128 fp32 matmul with a block-diagonal weight table computes 4 row-
    groups x 128 outputs at once (N=512).  Bias is fused into the PSUM->O
    copy as a tensor-tensor add against a broadcast bias tile.
  - because row % 64 == (64*p + k) % 64 == k, the PE half of O is the same
    for every group and every partition: it is written once at setup (device
    gather of the day table with `positions` as indices, bounced to DRAM,
    then replicated to all partitions by a partition-step-0 broadcast DMA)
    and never touched again; the store DMA re-reads it each group.
"""

import numpy as np
from contextlib import ExitStack

import concourse.bass as bass
import concourse.bacc as bacc
import concourse.mybir as mybir
import concourse.tile as tile
from concourse.bass_utils import run_bass_kernel_spmd
from concourse.masks import make_identity

F32 = mybir.dt.float32
I32 = mybir.dt.int32

N_CORES = 8
B, T, F, DH = 16384, 64, 16, 128
D = 2 * DH                       # 256
N_DAYS = 365                     # positions in [0, 365)
RPP = 64                         # rows per partition per group (must be = T)
GROUP = 128 * RPP                # 8192 rows per group
ROWS_PER_CORE = (B // N_CORES) * T   # 131072


def pe_day_table() -> np.ndarray:
    """Input-independent [365, 128] table: row d = interleaved
    sin/cos(d * inv_freq).  Mirrors reference arithmetic in float32."""
    i_even = np.arange(0, DH, 2, dtype=np.float32)
    inv_freq = np.power(
        np.float32(10000.0), -(np.float32(2.0) * i_even) / np.float32(DH)
    ).astype(np.float32)
    days = np.arange(N_DAYS, dtype=np.float32)
    ang = (days[:, None] * inv_freq[None, :]).astype(np.float32)
    tab = np.empty((N_DAYS, DH), dtype=np.float32)
    tab[:, 0::2] = np.sin(ang)
    tab[:, 1::2] = np.cos(ang)
    return tab


def build_program(rows: int = ROWS_PER_CORE):
    assert rows % GROUP == 0
    n_groups = rows // GROUP

    nc = bacc.Bacc("TRN2", target_bir_lowering=False, debug=False)

    feat = nc.dram_tensor("features", [rows, F], F32, kind="ExternalInput").ap()
    w_in = nc.dram_tensor("W", [DH, F], F32, kind="ExternalInput").ap()
    b_in = nc.dram_tensor("b", [DH], F32, kind="ExternalInput").ap()
    pos_in = nc.dram_tensor("positions", [T], I32, kind="ExternalInput").ap()
    tab_in = nc.dram_tensor("petab", [N_DAYS, DH], F32, kind="ExternalInput").ap()
    out = nc.dram_tensor("out", [rows, D], F32, kind="ExternalOutput").ap()
    pe_dram = nc.dram_tensor("pe_dram", [T * DH], F32).ap()  # internal scratch

    with tile.TileContext(nc) as tc, ExitStack() as ctx:
        const = ctx.enter_context(tc.tile_pool(name="const", bufs=1))
        psum = ctx.enter_context(tc.tile_pool(name="psum", bufs=1, space="PSUM"))
        ftp = ctx.enter_context(tc.tile_pool(name="ftp", bufs=6))
        tsp = ctx.enter_context(tc.tile_pool(name="tsp", bufs=3))
        opool = ctx.enter_context(tc.tile_pool(name="opool", bufs=1))

        feat_r = feat.rearrange("(g p r) f -> g p (r f)", p=128, r=RPP)
        out_r = out.rearrange("(g p r) d -> g p (r d)", p=128, r=RPP)

        # ---------- setup: constants ----------
        # PE warmup: a few junk matmuls issued at t=0 keep the PE busy long
        # enough (~3.4us) that the HAM clock-gate opens to 2.4 GHz before the
        # first real matmuls arrive (outputs are never read; memory writes are
        # not DCE'd).
        warm_src = const.tile([128, 512], F32)
        nc.vector.memset(warm_src[:], 0.0)
        for wi in range(6):
            warm_ps = psum.tile([128, 512], F32, tag="mm", bufs=5, name="warm_ps")
            nc.tensor.matmul(
                warm_ps[:],
                lhsT=warm_src[:, 0:128],
                rhs=warm_src[:],
                start=True,
                stop=True,
            )

        # issue the first feature load immediately so it overlaps setup
        ft0 = ftp.tile([128, RPP * F], F32, tag="ft", name="ft")
        nc.scalar.dma_start(ft0[:], feat_r[0])

        pos_sb = const.tile([T, 1], I32)
        nc.sync.dma_start(pos_sb[:], pos_in[:])
        w_sb = const.tile([DH, F], F32)
        nc.sync.dma_start(w_sb[:], w_in[:])
        b_sb = const.tile([1, DH], F32)
        nc.scalar.dma_start(b_sb[:], b_in[:])

        # gather pe[t] = petab[positions[t]] on device (embedding lookup),
        # then bounce through DRAM so a broadcast (partition-step-0) DMA can
        # replicate it to all 128 partitions of the staging buffers; first in
        # gpsimd program order so the chain starts as soon as pos_sb lands
        pe64 = const.tile([T, DH], F32)
        nc.gpsimd.indirect_dma_start(
            out=pe64[:],
            out_offset=None,
            in_=tab_in[:],
            in_offset=bass.IndirectOffsetOnAxis(ap=pos_sb[:, :1], axis=0),
        )
        nc.gpsimd.dma_start(pe_dram[:], pe64[:])

        ident = const.tile([128, 128], F32)
        make_identity(nc, ident)
        ones1 = const.tile([1, 128], F32)
        nc.gpsimd.memset(ones1[:], 1.0)

        # W [128,16] -> Wt [16,128] via PE transpose
        wt_ps = psum.tile([F, 128], F32, tag="tr", bufs=2)
        nc.tensor.transpose(wt_ps[:], w_sb[:], ident[:])
        wt_sb = const.tile([F, 128], F32)
        nc.vector.tensor_copy(wt_sb[:], wt_ps[:])

        # Block-diagonal weight tables [128, 512]:
        #   wta[16*j + f, 128*j + d] = Wt[f, d]   (row-groups 0..3)
        #   wtb[16*(j+4) + f, 128*j + d] = Wt[f, d]   (row-groups 4..7)
        wta = const.tile([128, 4 * DH], F32)
        wtb = const.tile([128, 4 * DH], F32)
        nc.vector.memset(wta[:], 0.0)
        nc.vector.memset(wtb[:], 0.0)
        # spread the partition-moving scatter copies over both HWDGE queues
        for j in range(4):
            nc.sync.dma_start(
                wta[16 * j : 16 * (j + 1), DH * j : DH * (j + 1)], wt_sb[:]
            )
            nc.scalar.dma_start(
                wtb[16 * (j + 4) : 16 * (j + 5), DH * j : DH * (j + 1)], wt_sb[:]
            )

        # bias broadcast [128, 512] (b tiled 4x along free dim)
        bias_ps = psum.tile([128, DH], F32, tag="tr", bufs=2)
        nc.tensor.matmul(bias_ps[:], lhsT=ones1[:], rhs=b_sb[:], start=True, stop=True)
        biasb = const.tile([128, 4 * DH], F32)
        for j in range(4):
            nc.vector.tensor_copy(biasb[:, DH * j : DH * (j + 1)], bias_ps[:])
        biasb3 = biasb.rearrange("p (j d) -> p j d", d=DH)

        # persistent output staging buffers (ping-pong); the pe columns are
        # written once (interleaved with group 0 below) and never re-written
        o_bufs = []
        for oi in range(2):
            o_t = opool.tile([128, RPP * D], F32, tag=f"o{oi}", name=f"obuf{oi}")
            o_bufs.append(o_t)

        # pe-column prefill: broadcast-DMA the gathered pe row block from DRAM
        # to every partition of O0 and O1.  These go on the gpsimd (SWDGE)
        # queue, which is otherwise idle — on the sync queue they would
        # head-of-line block group 0's piece stores (HWDGE queues drain FIFO).
        o_pe = [
            ob.rearrange("p (k d) -> p k d", d=D)[:, :, DH:D] for ob in o_bufs
        ]
        KQ = RPP // 4  # 16 k-blocks per prefill piece
        for q in range(4):
            src = bass.AP(
                pe_dram.tensor,
                KQ * DH * q,
                [[0, 128], [1, KQ * DH]],
            )
            nc.gpsimd.dma_start(o_pe[0][:, KQ * q : KQ * (q + 1), :], src)
            nc.gpsimd.dma_start(o_pe[1][:, KQ * q : KQ * (q + 1), :], src)

        # ---------- main loop ----------
        # group 0's 8 MiB store is split into 4 pieces of 2 MiB so the store
        # pipeline starts as soon as the first quarter is ready; later groups
        # use 2 pieces of 4 MiB (with sub-tile WAR tracking, the next group's
        # compute into a piece starts as soon as that piece's store drained).
        for g in range(n_groups):
            o_t = o_bufs[g % 2]
            o3 = o_t.rearrange("p (k d) -> p k d", d=D)
            if g == 0:
                ft = ft0
            else:
                ft = ftp.tile([128, RPP * F], F32, tag="ft", name="ft")
                nc.scalar.dma_start(ft[:], feat_r[g])
            npiece = 4 if g == 0 else 2
            cpp = 8 // npiece  # c-blocks per piece
            for q in range(npiece):
                for c in range(cpp * q, cpp * (q + 1)):
                    tr_ps = psum.tile([128, 128], F32, tag="tr", bufs=2, name="tr_ps")
                    nc.tensor.transpose(
                        tr_ps[:], ft[:, 128 * c : 128 * (c + 1)], ident[:]
                    )
                    t_sb = tsp.tile([128, 128], F32, tag="t", name="t_sb")
                    nc.vector.tensor_copy(t_sb[:], tr_ps[:])
                    for half, wblk in ((0, wta), (1, wtb)):
                        mm_ps = psum.tile(
                            [128, 512], F32, tag="mm", bufs=5, name="mm_ps"
                        )
                        nc.tensor.matmul(
                            mm_ps[:], lhsT=t_sb[:], rhs=wblk[:], start=True, stop=True
                        )
                        dst = o3[:, 8 * c + 4 * half : 8 * c + 4 * half + 4, 0:DH]
                        nc.vector.tensor_add(
                            dst, mm_ps.rearrange("p (j d) -> p j d", d=DH), biasb3
                        )
                lo = RPP * D * q // npiece
                hi = RPP * D * (q + 1) // npiece
                nc.sync.dma_start(out_r[g][:, lo:hi], o_t[:, lo:hi])

    nc.compile()
    return nc


_NC_CACHE: dict[int, object] = {}


def _get_program(rows: int):
    if rows not in _NC_CACHE:
        _NC_CACHE[rows] = build_program(rows)
    return _NC_CACHE[rows]


def kernel(features, W, b, positions, trace: bool = False, **run_kwargs):
    features = np.ascontiguousarray(np.asarray(features, dtype=np.float32))
    W = np.ascontiguousarray(np.asarray(W, dtype=np.float32))
    b = np.ascontiguousarray(np.asarray(b, dtype=np.float32))
    positions = np.ascontiguousarray(np.asarray(positions, dtype=np.int32))
    assert features.shape == (B, T, F)

    nc = _get_program(ROWS_PER_CORE)
    tab = pe_day_table()
    shards = features.reshape(N_CORES, ROWS_PER_CORE, F)
    in_maps = [
        {
            "features": shards[i],
            "W": W,
            "b": b,
            "positions": positions,
            "petab": tab,
        }
        for i in range(N_CORES)
    ]
    res = run_bass_kernel_spmd(
        nc, in_maps, core_ids=list(range(N_CORES)), trace=trace, **run_kwargs
    )
    out = np.concatenate(
        [res.results[i]["out"].reshape(B // N_CORES, T, D) for i in range(N_CORES)],
        axis=0,
    )
    if trace:
        kernel.last_results = res
    return out
